# revision 6
# baseline (speedup 1.0000x reference)
"""MoLoRA (mixture of LoRA experts with top-2 routing) Trainium2 Bass kernel.

Math (per token t, hidden H=640, experts E=5, rank R=8, router hidden 256):
  h      = silu(x @ W1 + b1)                 [T, 256]
  logits = h @ W2 + b2                       [T, 5]
  top-2 of softmax(logits), renormalized  == softmax over the top-2 logits:
     u1 = sigmoid(m1 - m2), u2 = sigmoid(m2 - m1)  (m1/m2 = top-2 logit values)
  dense weights w[t, e] = u1*[e==argmax1] + u2*[e==argmax2]  (0 elsewhere)
  low    = x @ Acat                          [T, 40]   (Acat[h,(e,r)] = A[e,h,r])
  delta  = (low * w_expanded) @ Bcat * 2.0   [T, 640]  (Bcat[(e,r),h] = Bm[e,r,h])
  out    = base_output + delta

Sharding: data-parallel over 8 NeuronCores; each core takes 4096 tokens
(= one batch row), router/LoRA params replicated.

All big matmuls (incl. router mm1) run in float32r at free-dim >= 256 so
the PE streams 1 row/cycle instead of fp32's 4; x transposes also run in
f32r PE mode (1.5 cyc/row).  base_output never enters a compute engine:
it is DMA-accumulated (SWDGE CCE-add) straight into the SBUF tile holding
delta.  The three per-tile transfers ride three different DMA queues
(x: SP HWDGE, store: ACT HWDGE, base-accum: gpsimd SWDGE).
"""

import numpy as np
from contextlib import ExitStack

import concourse.bass as bass
import concourse.tile as tile
from concourse import bacc
from concourse import mybir
from concourse.bass import ts
from concourse.masks import make_identity
from concourse.bass_utils import run_bass_kernel_spmd

F32 = mybir.dt.float32
F32R = mybir.dt.float32r
BF16 = mybir.dt.bfloat16
AF = mybir.ActivationFunctionType
ALU = mybir.AluOpType
AX = mybir.AxisListType

H = 640          # hidden
E = 5            # experts
R = 8            # lora rank
ER = E * R       # 40
RH = 256         # router hidden
HC = H // 128    # 5 h-chunks
RC = RH // 128   # 2 router-hidden chunks
SCALING = 16.0 / R
N_CORES = 8
T_CORE = 4096    # tokens per core (32768 / 8)
TT = 256         # token tile (2 halves of 128)


def build_kernel(t_core=T_CORE, niter=1, timing_mode=False, passes=1):
    assert t_core % TT == 0
    ntiles = t_core // TT
    nc = bacc.Bacc()

    if timing_mode:
        # big tensors stay on-device (uninitialized DRAM) so per-call wall
        # time isn't dominated by the axon host transfer; HBM traffic is
        # identical to the real kernel.
        x_d = nc.dram_tensor("x_int", [t_core, H], F32)[:, :]
        base_d = nc.dram_tensor("base_int", [t_core, H], F32)[:, :]
        out_d = nc.dram_tensor("out_int", [t_core, H], BF16)[:, :]
        dummy_d = nc.declare_dram_parameter("dummy_out", [1, 4], F32, isOutput=True)
    else:
        x_d = nc.declare_dram_parameter("x", [t_core, H], F32, isOutput=False)
        base_d = nc.declare_dram_parameter("base", [t_core, H], F32, isOutput=False)
        out_d = nc.declare_dram_parameter("out", [t_core, H], BF16, isOutput=True)
        dummy_d = None
    w1_d = nc.declare_dram_parameter("W1", [H, RH], F32, isOutput=False)
    b1_d = nc.declare_dram_parameter("b1", [RH], F32, isOutput=False)
    w2_d = nc.declare_dram_parameter("W2", [RH, E], F32, isOutput=False)
    b2_d = nc.declare_dram_parameter("b2", [E], F32, isOutput=False)
    a_d = nc.declare_dram_parameter("A", [E, H, R], F32, isOutput=False)
    bm_d = nc.declare_dram_parameter("Bm", [E, R, H], F32, isOutput=False)

    with ExitStack() as ctx:
        tc = ctx.enter_context(tile.TileContext(nc))
        const = ctx.enter_context(tc.tile_pool(name="const", bufs=1))
        xin_p = ctx.enter_context(tc.tile_pool(name="xin", bufs=4))
        bout_p = ctx.enter_context(tc.tile_pool(name="bout", bufs=6))
        xt_p = ctx.enter_context(tc.tile_pool(name="xt", bufs=3))
        ht_p = ctx.enter_context(tc.tile_pool(name="ht", bufs=3))
        small_p = ctx.enter_context(tc.tile_pool(name="small", bufs=6))
        lw_p = ctx.enter_context(tc.tile_pool(name="lw", bufs=4))
        ps_xt = ctx.enter_context(tc.tile_pool(name="ps_xt", bufs=1, space="PSUM"))
        ps_rt = ctx.enter_context(tc.tile_pool(name="ps_rt", bufs=1, space="PSUM"))
        ps_low = ctx.enter_context(tc.tile_pool(name="ps_low", bufs=1, space="PSUM"))
        ps_wrt = ctx.enter_context(tc.tile_pool(name="ps_wrt", bufs=1, space="PSUM"))
        ps_dl = ctx.enter_context(tc.tile_pool(name="ps_dl", bufs=1, space="PSUM"))

        # ---- constants / replicated params ----
        ident = const.tile([128, 128], F32)
        make_identity(nc, ident)
        ident_r = const.tile([128, 128], F32R)
        nc.vector.tensor_copy(out=ident_r, in_=ident)

        w1_sb = const.tile([128, HC, RH], F32R)
        nc.gpsimd.dma_start(
            out=w1_sb, in_=w1_d.bitcast(F32R).rearrange("(c p) m -> p c m", p=128)
        )
        b1_sb = const.tile([128, RC], F32)
        nc.gpsimd.dma_start(out=b1_sb, in_=b1_d.rearrange("(c p) -> p c", p=128))
        w2_sb = const.tile([128, RC, E], F32)
        nc.gpsimd.dma_start(out=w2_sb, in_=w2_d.rearrange("(c p) e -> p c e", p=128))
        b2_sb = const.tile([1, E], F32)
        nc.gpsimd.dma_start(out=b2_sb, in_=b2_d[:].unsqueeze(0))
        ones_sb = const.tile([1, 128], F32)
        nc.vector.memset(ones_sb, 1.0)
        # LoRA params concatenated over (e, r): index m = e*R + r.
        acat_sb = const.tile([128, HC, E, R], F32R)
        for e in range(E):
            for c in range(HC):
                nc.gpsimd.dma_start(
                    out=acat_sb[:, c, e, :],
                    in_=a_d.bitcast(F32R)[e, c * 128 : (c + 1) * 128, :],
                )
        bcat_sb = const.tile([ER, H], F32R)
        for e in range(E):
            nc.gpsimd.dma_start(
                out=bcat_sb[e * R : (e + 1) * R, :], in_=bm_d.bitcast(F32R)[e, :, :]
            )
        # NOTE: LoRA SCALING (=2.0) is folded into the lw multiply below.

        if dummy_d is not None:
            dnm = const.tile([1, 4], F32)
            nc.vector.memset(dnm, 1.0)
            nc.sync.dma_start(out=dummy_d[:, :], in_=dnm)

        loop_ctx = tc.For_i(0, niter, 1) if niter > 1 else None
        if loop_ctx is not None:
            ctx.enter_context(loop_ctx)

        def emit_front(i):
            """x load + xT transposes (f32r PE mode, single PSUM->SBUF copy)"""
            tok = i * TT
            x_nat = xin_p.tile([128, 2, H], F32R)
            nc.sync.dma_start(
                out=x_nat,
                in_=x_d.bitcast(F32R)[tok : tok + TT, :].rearrange(
                    "(j p) h -> p j h", p=128
                ),
            )

            xt_sb = xt_p.tile([128, HC, TT], F32R)
            for j in range(2):
                xtp = ps_xt.tile([128, HC, 128], F32R, tag="xtp")
                for c in range(HC):
                    nc.tensor.transpose(
                        out=xtp[:, c, :],
                        in_=x_nat[:, j, ts(c, 128)],
                        identity=ident_r,
                    )
                nc.any.tensor_copy(out=xt_sb[:, :, ts(j, 128)], in_=xtp)
            return {"xt_sb": xt_sb, "tok": tok}

        def emit_router(st):
            """mm1 -> silu -> (lowT interleaved) -> mm2 -> top-2 reduces"""
            xt_r = st["xt_sb"]
            # router mm1 in f32r at N=256: hT[rh, t] = (x @ W1)^T
            h_ps = ps_rt.tile([128, RC, TT], F32, tag="h")
            for c2 in range(RC):
                for c in range(HC):
                    nc.tensor.matmul(
                        out=h_ps[:, c2, :],
                        lhsT=w1_sb[:, c, ts(c2, 128)],
                        rhs=xt_r[:, c, :],
                        start=(c == 0),
                        stop=(c == HC - 1),
                    )
            # silu(z) = z * sigmoid(z), z = h + b1
            ht_sb = ht_p.tile([128, RC, TT], F32)
            sg_sb = ht_p.tile([128, RC, TT], F32, tag="sg")
            for c2 in range(RC):
                nc.scalar.activation(
                    out=sg_sb[:, c2, :],
                    in_=h_ps[:, c2, :],
                    func=AF.Sigmoid,
                    bias=b1_sb[:, c2 : c2 + 1],
                )
                nc.vector.scalar_tensor_tensor(
                    out=ht_sb[:, c2, :],
                    in0=h_ps[:, c2, :],
                    scalar=b1_sb[:, c2 : c2 + 1],
                    in1=sg_sb[:, c2, :],
                    op0=ALU.add,
                    op1=ALU.mult,
                )

            # lowT[(e,r), t] = (x @ Acat)^T  (f32r) -- emitted here so the
            # PE has work while ACT/DVE finish silu before mm2
            low_ps = ps_low.tile([ER, TT], F32, tag="low")
            for c in range(HC):
                nc.tensor.matmul(
                    out=low_ps,
                    lhsT=acat_sb[:, c, :, :],
                    rhs=xt_r[:, c, :],
                    start=(c == 0),
                    stop=(c == HC - 1),
                )
            st["low_ps"] = low_ps

            # router mm2 (token-major logits, exact fp32) + b2 via ones matmul
            lg_ps = ps_rt.tile([128, RC, E], F32, tag="lg")
            for j in range(2):
                for c2 in range(RC):
                    nc.tensor.matmul(
                        out=lg_ps[:, j, :],
                        lhsT=ht_sb[:, c2, ts(j, 128)],
                        rhs=w2_sb[:, c2, :],
                        start=(c2 == 0),
                        stop=False,
                    )
                nc.tensor.matmul(
                    out=lg_ps[:, j, :],
                    lhsT=ones_sb,
                    rhs=b2_sb,
                    start=False,
                    stop=True,
                )

            # top-2 selection, both token halves fused per op (j on free axis)
            m1 = small_p.tile([128, 2], F32, tag="m1")
            nc.vector.tensor_reduce(out=m1, in_=lg_ps, axis=AX.X, op=ALU.max)
            top1 = small_p.tile([128, 2, E], F32, tag="top1")
            nc.vector.tensor_tensor(
                out=top1,
                in0=lg_ps,
                in1=m1.unsqueeze(-1).broadcast_to([128, 2, E]),
                op=ALU.is_equal,
            )
            masked = small_p.tile([128, 2, E], F32, tag="masked")
            nc.vector.scalar_tensor_tensor(
                out=masked, in0=top1, scalar=-1e30, in1=lg_ps,
                op0=ALU.mult, op1=ALU.add,
            )
            m2 = small_p.tile([128, 2], F32, tag="m2")
            nc.vector.tensor_reduce(out=m2, in_=masked, axis=AX.X, op=ALU.max)
            dlg = small_p.tile([128, 2], F32, tag="dlg")
            nc.vector.tensor_tensor(out=dlg, in0=m2, in1=m1, op=ALU.subtract)
            st["top1"], st["masked"], st["m2"], st["dlg"] = top1, masked, m2, dlg

        def emit_weights(st):
            """softmax-over-top2 weights, expanded to (e,r)=40 via stride-0
            broadcast APs (one step later, so the ACT sigmoid never
            head-of-line blocks the next tile's copies)"""
            top1, masked, m2, dlg = st["top1"], st["masked"], st["m2"], st["dlg"]
            u2 = small_p.tile([128, 2], F32, tag="u2")
            nc.scalar.activation(out=u2, in_=dlg, func=AF.Sigmoid)
            u1 = small_p.tile([128, 2], F32, tag="u1")
            nc.vector.tensor_scalar(
                out=u1, in0=u2, scalar1=-1.0, scalar2=1.0,
                op0=ALU.mult, op1=ALU.add,
            )
            top2 = small_p.tile([128, 2, E], F32, tag="top2")
            nc.vector.tensor_tensor(
                out=top2,
                in0=masked,
                in1=m2.unsqueeze(-1).broadcast_to([128, 2, E]),
                op=ALU.is_equal,
            )
            # w_full[t, j, e*R+r] = u1*top1[e] + u2*top2[e]
            # (tile typed f32r: DVE writes round, feeding the f32r transpose)
            w_full = small_p.tile([128, 2, ER], F32R)
            wt2 = small_p.tile([128, 2, ER], F32, tag="wt2")
            w4 = w_full.rearrange("p j (e r) -> p j e r", r=R)
            wt24 = wt2.rearrange("p j (e r) -> p j e r", r=R)
            t14 = top1.unsqueeze(-1).broadcast_to([128, 2, E, R])
            t24 = top2.unsqueeze(-1).broadcast_to([128, 2, E, R])
            u14 = u1.unsqueeze(-1).unsqueeze(-1).broadcast_to([128, 2, E, R])
            u24 = u2.unsqueeze(-1).unsqueeze(-1).broadcast_to([128, 2, E, R])
            nc.vector.tensor_tensor(out=wt24, in0=t24, in1=u24, op=ALU.mult)
            nc.vector.tensor_tensor(out=w4, in0=t14, in1=u14, op=ALU.mult)
            nc.vector.tensor_tensor(out=w_full, in0=w_full, in1=wt2, op=ALU.add)
            st["w_full"] = w_full

        def emit_m(st):
            """middle: wrT transpose + weighted-low (feeds delta later)"""
            w_full = st["w_full"]
            wrt_ps = ps_wrt.tile([ER, 2, 128], F32R, tag="wrt")
            for j in range(2):
                nc.tensor.transpose(
                    out=wrt_ps[:, j, :],
                    in_=w_full[:, j, :],
                    identity=ident_r,
                )
            lw_sb = lw_p.tile([ER, TT], F32R)
            nc.vector.scalar_tensor_tensor(
                out=lw_sb,
                in0=st["low_ps"],
                scalar=float(SCALING),
                in1=wrt_ps.rearrange("p j t -> p (j t)"),
                op0=ALU.mult,
                op1=ALU.mult,
            )
            st["lw_sb"] = lw_sb

        def emit_b(st):
            """back half: delta matmuls, PSUM->SBUF copies, base_output
            DMA-accumulated into the tile, store"""
            lw_r, tok = st["lw_sb"], st["tok"]
            bcat_r = bcat_sb
            bo = bout_p.tile([128, 2, H], BF16)
            # delta in two 320-wide chunks (each >=256 keeps f32r at full
            # rate; each fits one PSUM bank in its own tile)
            for j in range(2):
                dla = ps_dl.tile([128, 320], F32, tag="dla")
                dlb = ps_dl.tile([128, 320], F32, tag="dlb")
                nc.tensor.matmul(
                    out=dla, lhsT=lw_r[:, ts(j, 128)], rhs=bcat_r[:, 0:320],
                    start=True, stop=True,
                )
                nc.tensor.matmul(
                    out=dlb, lhsT=lw_r[:, ts(j, 128)], rhs=bcat_r[:, 320:H],
                    start=True, stop=True,
                )
                nc.any.tensor_copy(out=bo[:, j, 0:320], in_=dla)
                nc.any.tensor_copy(out=bo[:, j, 320:H], in_=dlb)
            # bo += base_output, computed by the DMA engine (CCE add) while
            # it loads -- base never touches a compute engine
            nc.gpsimd.dma_start(
                out=bo,
                in_=base_d[tok : tok + TT, :].rearrange("(j p) h -> p j h", p=128),
                accum_op=ALU.add,
            )
            # store on the ACT HWDGE ring: SP keeps the x loads, gpsimd
            # keeps the base-accum loads -- one transfer per queue per tile
            nc.scalar.dma_start(
                out=out_d[tok : tok + TT, :].rearrange("(j p) h -> p j h", p=128),
                in_=bo,
            )

        prev = None
        for p in range(passes):
            for i in range(ntiles):
                st = emit_front(i)
                emit_router(st)
                emit_weights(st)
                emit_m(st)
                if prev is not None:
                    emit_b(prev)
                prev = st
        emit_b(prev)

    return nc


_CACHE = {}


def _get_nc(t_core=T_CORE, niter=1, timing_mode=False, passes=1):
    key = (t_core, niter, timing_mode, passes)
    if key not in _CACHE:
        nc = build_kernel(t_core, niter, timing_mode, passes)
        nc.finalize()
        _CACHE[key] = nc
    return _CACHE[key]


def kernel(x, base_output, W1, b1, W2, b2, A, Bm):
    x = np.ascontiguousarray(np.asarray(x), dtype=np.float32)
    base_output = np.ascontiguousarray(np.asarray(base_output), dtype=np.float32)
    W1 = np.ascontiguousarray(np.asarray(W1), dtype=np.float32)
    b1 = np.ascontiguousarray(np.asarray(b1), dtype=np.float32)
    W2 = np.ascontiguousarray(np.asarray(W2), dtype=np.float32)
    b2 = np.ascontiguousarray(np.asarray(b2), dtype=np.float32)
    A = np.ascontiguousarray(np.asarray(A), dtype=np.float32)
    Bm = np.ascontiguousarray(np.asarray(Bm), dtype=np.float32)

    B, S, _ = x.shape
    assert B * S == N_CORES * T_CORE
    xs = x.reshape(N_CORES, T_CORE, H)
    bs = base_output.reshape(N_CORES, T_CORE, H)

    nc = _get_nc()
    in_maps = [
        {
            "x": np.ascontiguousarray(xs[i]),
            "base": np.ascontiguousarray(bs[i]),
            "W1": W1, "b1": b1, "W2": W2, "b2": b2, "A": A, "Bm": Bm,
        }
        for i in range(N_CORES)
    ]
    res = run_bass_kernel_spmd(nc, in_maps, list(range(N_CORES))).results
    out = np.stack([res[i]["out"] for i in range(N_CORES)], axis=0)
    return out.reshape(B, S, H).astype(np.float32)


# revision 7
# speedup vs baseline: 1.2034x; 1.2034x over previous
"""MoLoRA (mixture of LoRA experts with top-2 routing) Trainium2 Bass kernel.

Math (per token t, hidden H=640, experts E=5, rank R=8, router hidden 256):
  h      = silu(x @ W1 + b1)                 [T, 256]
  logits = h @ W2 + b2                       [T, 5]
  top-2 of softmax(logits), renormalized  == softmax over the top-2 logits:
     u1 = sigmoid(m1 - m2), u2 = sigmoid(m2 - m1)  (m1/m2 = top-2 logit values)
  dense weights w[t, e] = u1*[e==argmax1] + u2*[e==argmax2]  (0 elsewhere)
  low    = x @ Acat                          [T, 40]   (Acat[h,(e,r)] = A[e,h,r])
  delta  = (low * w_expanded) @ Bcat * 2.0   [T, 640]  (Bcat[(e,r),h] = Bm[e,r,h])
  out    = base_output + delta

Sharding: data-parallel over 8 NeuronCores; each core takes 4096 tokens
(= one batch row), router/LoRA params replicated.

All big matmuls (incl. router mm1) run in float32r at free-dim >= 256 so
the PE streams 1 row/cycle instead of fp32's 4; x transposes also run in
f32r PE mode (1.5 cyc/row).  base_output never enters a compute engine:
it is DMA-accumulated (SWDGE CCE-add) straight into the SBUF tile holding
delta.  The three per-tile transfers ride three different DMA queues
(x: SP HWDGE, store: ACT HWDGE, base-accum: gpsimd SWDGE).
"""

import numpy as np
from contextlib import ExitStack

import concourse.bass as bass
import concourse.tile as tile
from concourse import bacc
from concourse import mybir
from concourse.bass import ts
from concourse.masks import make_identity
from concourse.bass_utils import run_bass_kernel_spmd

F32 = mybir.dt.float32
F32R = mybir.dt.float32r
BF16 = mybir.dt.bfloat16
AF = mybir.ActivationFunctionType
ALU = mybir.AluOpType
AX = mybir.AxisListType

H = 640          # hidden
E = 5            # experts
R = 8            # lora rank
ER = E * R       # 40
RH = 256         # router hidden
HC = H // 128    # 5 h-chunks
RC = RH // 128   # 2 router-hidden chunks
SCALING = 16.0 / R
N_CORES = 8
T_CORE = 4096    # tokens per core (32768 / 8)
TT = 256         # token tile (2 halves of 128)


def build_kernel(t_core=T_CORE, niter=1, timing_mode=False, passes=1):
    assert t_core % TT == 0
    ntiles = t_core // TT
    nc = bacc.Bacc()

    if timing_mode:
        # big tensors stay on-device (uninitialized DRAM) so per-call wall
        # time isn't dominated by the axon host transfer; HBM traffic is
        # identical to the real kernel.
        x_d = nc.dram_tensor("x_int", [t_core, H], F32)[:, :]
        base_d = nc.dram_tensor("base_int", [t_core, H], F32)[:, :]
        out_d = nc.dram_tensor("out_int", [t_core, H], F32)[:, :]
        dummy_d = nc.declare_dram_parameter("dummy_out", [1, 4], F32, isOutput=True)
    else:
        x_d = nc.declare_dram_parameter("x", [t_core, H], F32, isOutput=False)
        base_d = nc.declare_dram_parameter("base", [t_core, H], F32, isOutput=False)
        out_d = nc.declare_dram_parameter("out", [t_core, H], F32, isOutput=True)
        dummy_d = None
    w1_d = nc.declare_dram_parameter("W1", [H, RH], F32, isOutput=False)
    b1_d = nc.declare_dram_parameter("b1", [RH], F32, isOutput=False)
    w2_d = nc.declare_dram_parameter("W2", [RH, E], F32, isOutput=False)
    b2_d = nc.declare_dram_parameter("b2", [E], F32, isOutput=False)
    a_d = nc.declare_dram_parameter("A", [E, H, R], F32, isOutput=False)
    bm_d = nc.declare_dram_parameter("Bm", [E, R, H], F32, isOutput=False)

    with ExitStack() as ctx:
        tc = ctx.enter_context(tile.TileContext(nc))
        const = ctx.enter_context(tc.tile_pool(name="const", bufs=1))
        xin_p = ctx.enter_context(tc.tile_pool(name="xin", bufs=4))
        bout_p = ctx.enter_context(tc.tile_pool(name="bout", bufs=6))
        xt_p = ctx.enter_context(tc.tile_pool(name="xt", bufs=3))
        ht_p = ctx.enter_context(tc.tile_pool(name="ht", bufs=3))
        small_p = ctx.enter_context(tc.tile_pool(name="small", bufs=6))
        lw_p = ctx.enter_context(tc.tile_pool(name="lw", bufs=4))
        ps_xt = ctx.enter_context(tc.tile_pool(name="ps_xt", bufs=1, space="PSUM"))
        ps_rt = ctx.enter_context(tc.tile_pool(name="ps_rt", bufs=1, space="PSUM"))
        ps_low = ctx.enter_context(tc.tile_pool(name="ps_low", bufs=1, space="PSUM"))
        ps_wrt = ctx.enter_context(tc.tile_pool(name="ps_wrt", bufs=1, space="PSUM"))
        ps_dl = ctx.enter_context(tc.tile_pool(name="ps_dl", bufs=1, space="PSUM"))

        # ---- constants / replicated params ----
        ident = const.tile([128, 128], F32)
        make_identity(nc, ident)
        ident_r = const.tile([128, 128], F32R)
        nc.vector.tensor_copy(out=ident_r, in_=ident)

        w1_sb = const.tile([128, HC, RH], F32R)
        nc.gpsimd.dma_start(
            out=w1_sb, in_=w1_d.bitcast(F32R).rearrange("(c p) m -> p c m", p=128)
        )
        b1_sb = const.tile([128, RC], F32)
        nc.gpsimd.dma_start(out=b1_sb, in_=b1_d.rearrange("(c p) -> p c", p=128))
        w2_sb = const.tile([128, RC, E], F32)
        nc.gpsimd.dma_start(out=w2_sb, in_=w2_d.rearrange("(c p) e -> p c e", p=128))
        b2_sb = const.tile([1, E], F32)
        nc.gpsimd.dma_start(out=b2_sb, in_=b2_d[:].unsqueeze(0))
        ones_sb = const.tile([1, 128], F32)
        nc.vector.memset(ones_sb, 1.0)
        # LoRA params concatenated over (e, r): index m = e*R + r.
        acat_sb = const.tile([128, HC, E, R], F32R)
        for e in range(E):
            for c in range(HC):
                nc.gpsimd.dma_start(
                    out=acat_sb[:, c, e, :],
                    in_=a_d.bitcast(F32R)[e, c * 128 : (c + 1) * 128, :],
                )
        bcat_sb = const.tile([ER, H], F32R)
        for e in range(E):
            nc.gpsimd.dma_start(
                out=bcat_sb[e * R : (e + 1) * R, :], in_=bm_d.bitcast(F32R)[e, :, :]
            )
        # NOTE: LoRA SCALING (=2.0) is folded into the lw multiply below.

        if dummy_d is not None:
            dnm = const.tile([1, 4], F32)
            nc.vector.memset(dnm, 1.0)
            nc.sync.dma_start(out=dummy_d[:, :], in_=dnm)

        loop_ctx = tc.For_i(0, niter, 1) if niter > 1 else None
        if loop_ctx is not None:
            ctx.enter_context(loop_ctx)

        def emit_front(i):
            """x load + xT transposes (f32r PE mode, single PSUM->SBUF copy)"""
            tok = i * TT
            x_nat = xin_p.tile([128, 2, H], F32R)
            nc.sync.dma_start(
                out=x_nat,
                in_=x_d.bitcast(F32R)[tok : tok + TT, :].rearrange(
                    "(j p) h -> p j h", p=128
                ),
            )

            xt_sb = xt_p.tile([128, HC, TT], F32R)
            for j in range(2):
                xtp = ps_xt.tile([128, HC, 128], F32R, tag="xtp")
                for c in range(HC):
                    nc.tensor.transpose(
                        out=xtp[:, c, :],
                        in_=x_nat[:, j, ts(c, 128)],
                        identity=ident_r,
                    )
                nc.any.tensor_copy(out=xt_sb[:, :, ts(j, 128)], in_=xtp)
            return {"xt_sb": xt_sb, "tok": tok}

        def emit_router(st):
            """mm1 -> silu -> (lowT interleaved) -> mm2 -> top-2 reduces"""
            xt_r = st["xt_sb"]
            # router mm1 in f32r at N=256: hT[rh, t] = (x @ W1)^T
            h_ps = ps_rt.tile([128, RC, TT], F32, tag="h")
            for c2 in range(RC):
                for c in range(HC):
                    nc.tensor.matmul(
                        out=h_ps[:, c2, :],
                        lhsT=w1_sb[:, c, ts(c2, 128)],
                        rhs=xt_r[:, c, :],
                        start=(c == 0),
                        stop=(c == HC - 1),
                    )
            # silu(z) = z * sigmoid(z), z = h + b1
            ht_sb = ht_p.tile([128, RC, TT], F32)
            sg_sb = ht_p.tile([128, RC, TT], F32, tag="sg")
            for c2 in range(RC):
                nc.scalar.activation(
                    out=sg_sb[:, c2, :],
                    in_=h_ps[:, c2, :],
                    func=AF.Sigmoid,
                    bias=b1_sb[:, c2 : c2 + 1],
                )
                nc.vector.scalar_tensor_tensor(
                    out=ht_sb[:, c2, :],
                    in0=h_ps[:, c2, :],
                    scalar=b1_sb[:, c2 : c2 + 1],
                    in1=sg_sb[:, c2, :],
                    op0=ALU.add,
                    op1=ALU.mult,
                )

            # lowT[(e,r), t] = (x @ Acat)^T  (f32r) -- emitted here so the
            # PE has work while ACT/DVE finish silu before mm2
            low_ps = ps_low.tile([ER, TT], F32, tag="low")
            for c in range(HC):
                nc.tensor.matmul(
                    out=low_ps,
                    lhsT=acat_sb[:, c, :, :],
                    rhs=xt_r[:, c, :],
                    start=(c == 0),
                    stop=(c == HC - 1),
                )
            st["low_ps"] = low_ps

            # router mm2 (token-major logits, exact fp32) + b2 via ones matmul
            lg_ps = ps_rt.tile([128, RC, E], F32, tag="lg")
            for j in range(2):
                for c2 in range(RC):
                    nc.tensor.matmul(
                        out=lg_ps[:, j, :],
                        lhsT=ht_sb[:, c2, ts(j, 128)],
                        rhs=w2_sb[:, c2, :],
                        start=(c2 == 0),
                        stop=False,
                    )
                nc.tensor.matmul(
                    out=lg_ps[:, j, :],
                    lhsT=ones_sb,
                    rhs=b2_sb,
                    start=False,
                    stop=True,
                )

            # top-2 selection, both token halves fused per op (j on free axis)
            m1 = small_p.tile([128, 2], F32, tag="m1")
            nc.vector.tensor_reduce(out=m1, in_=lg_ps, axis=AX.X, op=ALU.max)
            top1 = small_p.tile([128, 2, E], F32, tag="top1")
            nc.vector.tensor_tensor(
                out=top1,
                in0=lg_ps,
                in1=m1.unsqueeze(-1).broadcast_to([128, 2, E]),
                op=ALU.is_equal,
            )
            masked = small_p.tile([128, 2, E], F32, tag="masked")
            nc.vector.scalar_tensor_tensor(
                out=masked, in0=top1, scalar=-1e30, in1=lg_ps,
                op0=ALU.mult, op1=ALU.add,
            )
            m2 = small_p.tile([128, 2], F32, tag="m2")
            nc.vector.tensor_reduce(out=m2, in_=masked, axis=AX.X, op=ALU.max)
            dlg = small_p.tile([128, 2], F32, tag="dlg")
            nc.vector.tensor_tensor(out=dlg, in0=m2, in1=m1, op=ALU.subtract)
            st["top1"], st["masked"], st["m2"], st["dlg"] = top1, masked, m2, dlg

        def emit_weights(st):
            """softmax-over-top2 weights, expanded to (e,r)=40 via stride-0
            broadcast APs (one step later, so the ACT sigmoid never
            head-of-line blocks the next tile's copies)"""
            top1, masked, m2, dlg = st["top1"], st["masked"], st["m2"], st["dlg"]
            u2 = small_p.tile([128, 2], F32, tag="u2")
            nc.scalar.activation(out=u2, in_=dlg, func=AF.Sigmoid)
            u1 = small_p.tile([128, 2], F32, tag="u1")
            nc.vector.tensor_scalar(
                out=u1, in0=u2, scalar1=-1.0, scalar2=1.0,
                op0=ALU.mult, op1=ALU.add,
            )
            top2 = small_p.tile([128, 2, E], F32, tag="top2")
            nc.vector.tensor_tensor(
                out=top2,
                in0=masked,
                in1=m2.unsqueeze(-1).broadcast_to([128, 2, E]),
                op=ALU.is_equal,
            )
            # w_full[t, j, e*R+r] = u1*top1[e] + u2*top2[e]
            # (tile typed f32r: DVE writes round, feeding the f32r transpose)
            w_full = small_p.tile([128, 2, ER], F32R)
            wt2 = small_p.tile([128, 2, ER], F32, tag="wt2")
            w4 = w_full.rearrange("p j (e r) -> p j e r", r=R)
            wt24 = wt2.rearrange("p j (e r) -> p j e r", r=R)
            t14 = top1.unsqueeze(-1).broadcast_to([128, 2, E, R])
            t24 = top2.unsqueeze(-1).broadcast_to([128, 2, E, R])
            u14 = u1.unsqueeze(-1).unsqueeze(-1).broadcast_to([128, 2, E, R])
            u24 = u2.unsqueeze(-1).unsqueeze(-1).broadcast_to([128, 2, E, R])
            nc.vector.tensor_tensor(out=wt24, in0=t24, in1=u24, op=ALU.mult)
            nc.vector.tensor_tensor(out=w4, in0=t14, in1=u14, op=ALU.mult)
            nc.vector.tensor_tensor(out=w_full, in0=w_full, in1=wt2, op=ALU.add)
            st["w_full"] = w_full

        def emit_m(st):
            """middle: wrT transpose + weighted-low (feeds delta later)"""
            w_full = st["w_full"]
            wrt_ps = ps_wrt.tile([ER, 2, 128], F32R, tag="wrt")
            for j in range(2):
                nc.tensor.transpose(
                    out=wrt_ps[:, j, :],
                    in_=w_full[:, j, :],
                    identity=ident_r,
                )
            wrt_sb = small_p.tile([ER, 2, 128], F32R, tag="wrt_sb")
            nc.any.tensor_copy(out=wrt_sb, in_=wrt_ps)
            lw_sb = lw_p.tile([ER, TT], F32R)
            nc.vector.scalar_tensor_tensor(
                out=lw_sb,
                in0=st["low_ps"],
                scalar=float(SCALING),
                in1=wrt_sb.rearrange("p j t -> p (j t)"),
                op0=ALU.mult,
                op1=ALU.mult,
            )
            st["lw_sb"] = lw_sb

        def emit_b(st):
            """back half: delta matmuls, PSUM->SBUF copies, base_output
            DMA-accumulated into the tile, store"""
            lw_r, tok = st["lw_sb"], st["tok"]
            bcat_r = bcat_sb
            bo = bout_p.tile([128, 2, H], F32)
            # delta in two 320-wide chunks (each >=256 keeps f32r at full
            # rate; each fits one PSUM bank in its own tile)
            for j in range(2):
                dla = ps_dl.tile([128, 320], F32, tag="dla")
                dlb = ps_dl.tile([128, 320], F32, tag="dlb")
                nc.tensor.matmul(
                    out=dla, lhsT=lw_r[:, ts(j, 128)], rhs=bcat_r[:, 0:320],
                    start=True, stop=True,
                )
                nc.tensor.matmul(
                    out=dlb, lhsT=lw_r[:, ts(j, 128)], rhs=bcat_r[:, 320:H],
                    start=True, stop=True,
                )
                nc.any.tensor_copy(out=bo[:, j, 0:320], in_=dla)
                nc.any.tensor_copy(out=bo[:, j, 320:H], in_=dlb)
            # bo += base_output, computed by the DMA engine (CCE add) while
            # it loads -- base never touches a compute engine
            nc.gpsimd.dma_start(
                out=bo,
                in_=base_d[tok : tok + TT, :].rearrange("(j p) h -> p j h", p=128),
                accum_op=ALU.add,
            )
            # store on the ACT HWDGE ring: SP keeps the x loads, gpsimd
            # keeps the base-accum loads -- one transfer per queue per tile
            nc.scalar.dma_start(
                out=out_d[tok : tok + TT, :].rearrange("(j p) h -> p j h", p=128),
                in_=bo,
            )

        prev = None
        for p in range(passes):
            for i in range(ntiles):
                st = emit_front(i)
                emit_router(st)
                emit_weights(st)
                emit_m(st)
                if prev is not None:
                    emit_b(prev)
                prev = st
        emit_b(prev)

    return nc


_CACHE = {}


def _get_nc(t_core=T_CORE, niter=1, timing_mode=False, passes=1):
    key = (t_core, niter, timing_mode, passes)
    if key not in _CACHE:
        nc = build_kernel(t_core, niter, timing_mode, passes)
        nc.finalize()
        _CACHE[key] = nc
    return _CACHE[key]


def kernel(x, base_output, W1, b1, W2, b2, A, Bm):
    x = np.ascontiguousarray(np.asarray(x), dtype=np.float32)
    base_output = np.ascontiguousarray(np.asarray(base_output), dtype=np.float32)
    W1 = np.ascontiguousarray(np.asarray(W1), dtype=np.float32)
    b1 = np.ascontiguousarray(np.asarray(b1), dtype=np.float32)
    W2 = np.ascontiguousarray(np.asarray(W2), dtype=np.float32)
    b2 = np.ascontiguousarray(np.asarray(b2), dtype=np.float32)
    A = np.ascontiguousarray(np.asarray(A), dtype=np.float32)
    Bm = np.ascontiguousarray(np.asarray(Bm), dtype=np.float32)

    B, S, _ = x.shape
    assert B * S == N_CORES * T_CORE
    xs = x.reshape(N_CORES, T_CORE, H)
    bs = base_output.reshape(N_CORES, T_CORE, H)

    nc = _get_nc()
    in_maps = [
        {
            "x": np.ascontiguousarray(xs[i]),
            "base": np.ascontiguousarray(bs[i]),
            "W1": W1, "b1": b1, "W2": W2, "b2": b2, "A": A, "Bm": Bm,
        }
        for i in range(N_CORES)
    ]
    res = run_bass_kernel_spmd(nc, in_maps, list(range(N_CORES))).results
    out = np.stack([res[i]["out"] for i in range(N_CORES)], axis=0)
    return out.reshape(B, S, H).astype(np.float32)


# revision 8
# speedup vs baseline: 1.4336x; 1.1913x over previous
"""MoLoRA (mixture of LoRA experts with top-2 routing) Trainium2 Bass kernel.

Math (per token t, hidden H=640, experts E=5, rank R=8, router hidden 256):
  h      = silu(x @ W1 + b1)                 [T, 256]
  logits = h @ W2 + b2                       [T, 5]
  top-2 of softmax(logits), renormalized  == softmax over the top-2 logits:
     u1 = sigmoid(m1 - m2), u2 = sigmoid(m2 - m1)  (m1/m2 = top-2 logit values)
  dense weights w[t, e] = u1*[e==argmax1] + u2*[e==argmax2]  (0 elsewhere)
  low    = x @ Acat                          [T, 40]   (Acat[h,(e,r)] = A[e,h,r])
  delta  = (low * w_expanded) @ Bcat * 2.0   [T, 640]  (Bcat[(e,r),h] = Bm[e,r,h])
  out    = base_output + delta

Sharding: data-parallel over 8 NeuronCores; each core takes 4096 tokens
(= one batch row), router/LoRA params replicated.

All big matmuls (incl. router mm1) run in float32r at free-dim >= 256 so
the PE streams 1 row/cycle instead of fp32's 4; x transposes also run in
f32r PE mode (1.5 cyc/row).  base_output never enters a compute engine:
it is DMA-accumulated (SWDGE CCE-add) straight into the SBUF tile holding
delta.  The three per-tile transfers ride three different DMA queues
(x: SP HWDGE, store: ACT HWDGE, base-accum: gpsimd SWDGE).
"""

import numpy as np
from contextlib import ExitStack

import concourse.bass as bass
import concourse.tile as tile
from concourse import bacc
from concourse import mybir
from concourse.bass import ts
from concourse.masks import make_identity
from concourse.bass_utils import run_bass_kernel_spmd

F32 = mybir.dt.float32
F32R = mybir.dt.float32r
BF16 = mybir.dt.bfloat16
AF = mybir.ActivationFunctionType
ALU = mybir.AluOpType
AX = mybir.AxisListType

H = 640          # hidden
E = 5            # experts
R = 8            # lora rank
ER = E * R       # 40
RH = 256         # router hidden
HC = H // 128    # 5 h-chunks
RC = RH // 128   # 2 router-hidden chunks
SCALING = 16.0 / R
N_CORES = 8
T_CORE = 4096    # tokens per core (32768 / 8)
TT = 256         # token tile (2 halves of 128)


def build_kernel(t_core=T_CORE, niter=1, timing_mode=False, passes=1):
    assert t_core % TT == 0
    ntiles = t_core // TT
    nc = bacc.Bacc()

    if timing_mode:
        # big tensors stay on-device (uninitialized DRAM) so per-call wall
        # time isn't dominated by the axon host transfer; HBM traffic is
        # identical to the real kernel.
        x_d = nc.dram_tensor("x_int", [t_core, H], F32)[:, :]
        base_d = nc.dram_tensor("base_int", [t_core, H], F32)[:, :]
        out_d = nc.dram_tensor("out_int", [t_core, H], BF16)[:, :]
        dummy_d = nc.declare_dram_parameter("dummy_out", [1, 4], F32, isOutput=True)
    else:
        x_d = nc.declare_dram_parameter("x", [t_core, H], F32, isOutput=False)
        base_d = nc.declare_dram_parameter("base", [t_core, H], F32, isOutput=False)
        out_d = nc.declare_dram_parameter("out", [t_core, H], BF16, isOutput=True)
        dummy_d = None
    w1_d = nc.declare_dram_parameter("W1", [H, RH], F32, isOutput=False)
    b1_d = nc.declare_dram_parameter("b1", [RH], F32, isOutput=False)
    w2_d = nc.declare_dram_parameter("W2", [RH, E], F32, isOutput=False)
    b2_d = nc.declare_dram_parameter("b2", [E], F32, isOutput=False)
    a_d = nc.declare_dram_parameter("A", [E, H, R], F32, isOutput=False)
    bm_d = nc.declare_dram_parameter("Bm", [E, R, H], F32, isOutput=False)

    with ExitStack() as ctx:
        tc = ctx.enter_context(tile.TileContext(nc))
        const = ctx.enter_context(tc.tile_pool(name="const", bufs=1))
        xin_p = ctx.enter_context(tc.tile_pool(name="xin", bufs=3))
        bout_p = ctx.enter_context(tc.tile_pool(name="bout", bufs=4))
        base_p = ctx.enter_context(tc.tile_pool(name="basep", bufs=3))
        xt_p = ctx.enter_context(tc.tile_pool(name="xt", bufs=2))
        ht_p = ctx.enter_context(tc.tile_pool(name="ht", bufs=2))
        small_p = ctx.enter_context(tc.tile_pool(name="small", bufs=4))
        lw_p = ctx.enter_context(tc.tile_pool(name="lw", bufs=3))
        ps_xt = ctx.enter_context(tc.tile_pool(name="ps_xt", bufs=1, space="PSUM"))
        ps_rt = ctx.enter_context(tc.tile_pool(name="ps_rt", bufs=1, space="PSUM"))
        ps_low = ctx.enter_context(tc.tile_pool(name="ps_low", bufs=2, space="PSUM"))
        ps_wrt = ctx.enter_context(tc.tile_pool(name="ps_wrt", bufs=1, space="PSUM"))
        ps_dl = ctx.enter_context(tc.tile_pool(name="ps_dl", bufs=1, space="PSUM"))

        # ---- constants / replicated params ----
        ident = const.tile([128, 128], F32)
        make_identity(nc, ident)
        ident_r = const.tile([128, 128], F32R)
        nc.vector.tensor_copy(out=ident_r, in_=ident)

        w1_sb = const.tile([128, HC, RH], F32R)
        nc.gpsimd.dma_start(
            out=w1_sb, in_=w1_d.bitcast(F32R).rearrange("(c p) m -> p c m", p=128)
        )
        b1_sb = const.tile([128, RC], F32)
        nc.gpsimd.dma_start(out=b1_sb, in_=b1_d.rearrange("(c p) -> p c", p=128))
        w2_sb = const.tile([128, RC, E], F32)
        nc.gpsimd.dma_start(out=w2_sb, in_=w2_d.rearrange("(c p) e -> p c e", p=128))
        b2_sb = const.tile([1, E], F32)
        nc.gpsimd.dma_start(out=b2_sb, in_=b2_d[:].unsqueeze(0))
        ones_sb = const.tile([1, 128], F32)
        nc.vector.memset(ones_sb, 1.0)
        # LoRA params concatenated over (e, r): index m = e*R + r.
        acat_sb = const.tile([128, HC, E, R], F32R)
        for e in range(E):
            for c in range(HC):
                nc.gpsimd.dma_start(
                    out=acat_sb[:, c, e, :],
                    in_=a_d.bitcast(F32R)[e, c * 128 : (c + 1) * 128, :],
                )
        bcat_sb = const.tile([ER, H], F32R)
        for e in range(E):
            nc.gpsimd.dma_start(
                out=bcat_sb[e * R : (e + 1) * R, :], in_=bm_d.bitcast(F32R)[e, :, :]
            )
        # NOTE: LoRA SCALING (=2.0) is folded into the lw multiply below.

        if dummy_d is not None:
            dnm = const.tile([1, 4], F32)
            nc.vector.memset(dnm, 1.0)
            nc.sync.dma_start(out=dummy_d[:, :], in_=dnm)

        loop_ctx = tc.For_i(0, niter, 1) if niter > 1 else None
        if loop_ctx is not None:
            ctx.enter_context(loop_ctx)

        def emit_front(i):
            """x load + xT transposes (f32r PE mode, single PSUM->SBUF copy)"""
            tok = i * TT
            x_nat = xin_p.tile([128, 2, H], F32R)
            nc.sync.dma_start(
                out=x_nat,
                in_=x_d.bitcast(F32R)[tok : tok + TT, :].rearrange(
                    "(j p) h -> p j h", p=128
                ),
            )

            base_sb = base_p.tile([128, 2, H], F32)
            nc.gpsimd.dma_start(
                out=base_sb,
                in_=base_d[tok : tok + TT, :].rearrange("(j p) h -> p j h", p=128),
            )

            xt_sb = xt_p.tile([128, HC, TT], F32R)
            for j in range(2):
                xtp = ps_xt.tile([128, HC, 128], F32R, tag="xtp")
                for c in range(HC):
                    nc.tensor.transpose(
                        out=xtp[:, c, :],
                        in_=x_nat[:, j, ts(c, 128)],
                        identity=ident_r,
                    )
                nc.any.tensor_copy(out=xt_sb[:, :, ts(j, 128)], in_=xtp)
            return {"xt_sb": xt_sb, "base_sb": base_sb, "tok": tok}

        def emit_router(st):
            """mm1 -> silu -> (lowT interleaved) -> mm2 -> top-2 reduces"""
            xt_r = st["xt_sb"]
            # router mm1 in f32r at N=256: hT[rh, t] = (x @ W1)^T
            h_ps = ps_rt.tile([128, RC, TT], F32, tag="rt")
            for c2 in range(RC):
                for c in range(HC):
                    nc.tensor.matmul(
                        out=h_ps[:, c2, :],
                        lhsT=w1_sb[:, c, ts(c2, 128)],
                        rhs=xt_r[:, c, :],
                        start=(c == 0),
                        stop=(c == HC - 1),
                    )
            # silu(z) = z * sigmoid(z), z = h + b1
            ht_sb = ht_p.tile([128, RC, TT], F32)
            sg_sb = ht_p.tile([128, RC, TT], F32, tag="sg")
            for c2 in range(RC):
                nc.scalar.activation(
                    out=sg_sb[:, c2, :],
                    in_=h_ps[:, c2, :],
                    func=AF.Sigmoid,
                    bias=b1_sb[:, c2 : c2 + 1],
                )
                nc.vector.scalar_tensor_tensor(
                    out=ht_sb[:, c2, :],
                    in0=h_ps[:, c2, :],
                    scalar=b1_sb[:, c2 : c2 + 1],
                    in1=sg_sb[:, c2, :],
                    op0=ALU.add,
                    op1=ALU.mult,
                )

            # lowT[(e,r), t] = (x @ Acat)^T  (f32r) -- emitted here so the
            # PE has work while ACT/DVE finish silu before mm2
            low_ps = ps_low.tile([ER, TT], F32, tag="low")
            for c in range(HC):
                nc.tensor.matmul(
                    out=low_ps,
                    lhsT=acat_sb[:, c, :, :],
                    rhs=xt_r[:, c, :],
                    start=(c == 0),
                    stop=(c == HC - 1),
                )
            st["low_ps"] = low_ps

            # router mm2 (token-major logits, exact fp32) + b2 via ones matmul
            lg_full = ps_rt.tile([128, RC, TT], F32, tag="rt")
            lg_ps = lg_full[:, :, 0:E]
            for j in range(2):
                for c2 in range(RC):
                    nc.tensor.matmul(
                        out=lg_ps[:, j, :],
                        lhsT=ht_sb[:, c2, ts(j, 128)],
                        rhs=w2_sb[:, c2, :],
                        start=(c2 == 0),
                        stop=False,
                    )
                nc.tensor.matmul(
                    out=lg_ps[:, j, :],
                    lhsT=ones_sb,
                    rhs=b2_sb,
                    start=False,
                    stop=True,
                )

            # top-2 selection, both token halves fused per op (j on free axis)
            m1 = small_p.tile([128, 2], F32, tag="m1")
            nc.vector.tensor_reduce(out=m1, in_=lg_ps, axis=AX.X, op=ALU.max)
            top1 = small_p.tile([128, 2, E], F32, tag="top1")
            nc.vector.tensor_tensor(
                out=top1,
                in0=lg_ps,
                in1=m1.unsqueeze(-1).broadcast_to([128, 2, E]),
                op=ALU.is_equal,
            )
            masked = small_p.tile([128, 2, E], F32, tag="masked")
            nc.vector.scalar_tensor_tensor(
                out=masked, in0=top1, scalar=-1e30, in1=lg_ps,
                op0=ALU.mult, op1=ALU.add,
            )
            m2 = small_p.tile([128, 2], F32, tag="m2")
            nc.vector.tensor_reduce(out=m2, in_=masked, axis=AX.X, op=ALU.max)
            dlg = small_p.tile([128, 2], F32, tag="dlg")
            nc.vector.tensor_tensor(out=dlg, in0=m2, in1=m1, op=ALU.subtract)
            st["top1"], st["masked"], st["m2"], st["dlg"] = top1, masked, m2, dlg

        def emit_weights(st):
            """softmax-over-top2 weights, expanded to (e,r)=40 via stride-0
            broadcast APs (one step later, so the ACT sigmoid never
            head-of-line blocks the next tile's copies)"""
            top1, masked, m2, dlg = st["top1"], st["masked"], st["m2"], st["dlg"]
            u2 = small_p.tile([128, 2], F32, tag="u2")
            nc.scalar.activation(out=u2, in_=dlg, func=AF.Sigmoid)
            u1 = small_p.tile([128, 2], F32, tag="u1")
            nc.vector.tensor_scalar(
                out=u1, in0=u2, scalar1=-1.0, scalar2=1.0,
                op0=ALU.mult, op1=ALU.add,
            )
            top2 = small_p.tile([128, 2, E], F32, tag="top2")
            nc.vector.tensor_tensor(
                out=top2,
                in0=masked,
                in1=m2.unsqueeze(-1).broadcast_to([128, 2, E]),
                op=ALU.is_equal,
            )
            # w_full[t, j, e*R+r] = u1*top1[e] + u2*top2[e]
            # (tile typed f32r: DVE writes round, feeding the f32r transpose)
            w_full = small_p.tile([128, 2, ER], F32R)
            wt2 = small_p.tile([128, 2, ER], F32, tag="wt2")
            w4 = w_full.rearrange("p j (e r) -> p j e r", r=R)
            wt24 = wt2.rearrange("p j (e r) -> p j e r", r=R)
            t14 = top1.unsqueeze(-1).broadcast_to([128, 2, E, R])
            t24 = top2.unsqueeze(-1).broadcast_to([128, 2, E, R])
            u14 = u1.unsqueeze(-1).unsqueeze(-1).broadcast_to([128, 2, E, R])
            u24 = u2.unsqueeze(-1).unsqueeze(-1).broadcast_to([128, 2, E, R])
            nc.vector.tensor_tensor(out=wt24, in0=t24, in1=u24, op=ALU.mult)
            nc.vector.tensor_tensor(out=w4, in0=t14, in1=u14, op=ALU.mult)
            nc.vector.tensor_tensor(out=w_full, in0=w_full, in1=wt2, op=ALU.add)
            st["w_full"] = w_full

        def emit_m(st):
            """middle: wrT transpose + weighted-low (feeds delta later)"""
            w_full = st["w_full"]
            wrt_ps = ps_wrt.tile([ER, 2, 128], F32R, tag="wrt")
            for j in range(2):
                nc.tensor.transpose(
                    out=wrt_ps[:, j, :],
                    in_=w_full[:, j, :],
                    identity=ident_r,
                )
            wrt_sb = small_p.tile([ER, 2, 128], F32R, tag="wrt_sb")
            nc.any.tensor_copy(out=wrt_sb, in_=wrt_ps)
            lw_sb = lw_p.tile([ER, TT], F32R)
            nc.vector.scalar_tensor_tensor(
                out=lw_sb,
                in0=st["low_ps"],
                scalar=float(SCALING),
                in1=wrt_sb.rearrange("p j t -> p (j t)"),
                op0=ALU.mult,
                op1=ALU.mult,
            )
            st["lw_sb"] = lw_sb

        def emit_b(st):
            """back half: delta matmuls, PSUM->SBUF copies, base_output
            DMA-accumulated into the tile, store"""
            lw_r, tok = st["lw_sb"], st["tok"]
            bcat_r = bcat_sb
            bo = bout_p.tile([128, 2, H], BF16)
            base_sb = st["base_sb"]
            # delta in two 320-wide chunks (each >=256 keeps f32r at full
            # rate; each fits one PSUM bank in its own tile); the PSUM->SBUF
            # move fuses the base add and the bf16 downcast
            for j in range(2):
                dla = ps_dl.tile([128, 320], F32, tag="dla")
                dlb = ps_dl.tile([128, 320], F32, tag="dlb")
                nc.tensor.matmul(
                    out=dla, lhsT=lw_r[:, ts(j, 128)], rhs=bcat_r[:, 0:320],
                    start=True, stop=True,
                )
                nc.tensor.matmul(
                    out=dlb, lhsT=lw_r[:, ts(j, 128)], rhs=bcat_r[:, 320:H],
                    start=True, stop=True,
                )
                nc.any.tensor_tensor(
                    out=bo[:, j, 0:320], in0=dla, in1=base_sb[:, j, 0:320],
                    op=ALU.add,
                )
                nc.any.tensor_tensor(
                    out=bo[:, j, 320:H], in0=dlb, in1=base_sb[:, j, 320:H],
                    op=ALU.add,
                )
            # store (bf16, half the bytes) on the ACT HWDGE ring: SP keeps
            # the x loads, gpsimd the base loads -- one transfer per queue
            nc.scalar.dma_start(
                out=out_d[tok : tok + TT, :].rearrange("(j p) h -> p j h", p=128),
                in_=bo,
            )

        prev = None
        for p in range(passes):
            for i in range(ntiles):
                st = emit_front(i)
                emit_router(st)
                emit_weights(st)
                emit_m(st)
                if prev is not None:
                    emit_b(prev)
                prev = st
        emit_b(prev)

    return nc


_CACHE = {}


def _get_nc(t_core=T_CORE, niter=1, timing_mode=False, passes=1):
    key = (t_core, niter, timing_mode, passes)
    if key not in _CACHE:
        nc = build_kernel(t_core, niter, timing_mode, passes)
        nc.finalize()
        _CACHE[key] = nc
    return _CACHE[key]


def kernel(x, base_output, W1, b1, W2, b2, A, Bm):
    x = np.ascontiguousarray(np.asarray(x), dtype=np.float32)
    base_output = np.ascontiguousarray(np.asarray(base_output), dtype=np.float32)
    W1 = np.ascontiguousarray(np.asarray(W1), dtype=np.float32)
    b1 = np.ascontiguousarray(np.asarray(b1), dtype=np.float32)
    W2 = np.ascontiguousarray(np.asarray(W2), dtype=np.float32)
    b2 = np.ascontiguousarray(np.asarray(b2), dtype=np.float32)
    A = np.ascontiguousarray(np.asarray(A), dtype=np.float32)
    Bm = np.ascontiguousarray(np.asarray(Bm), dtype=np.float32)

    B, S, _ = x.shape
    assert B * S == N_CORES * T_CORE
    xs = x.reshape(N_CORES, T_CORE, H)
    bs = base_output.reshape(N_CORES, T_CORE, H)

    nc = _get_nc()
    in_maps = [
        {
            "x": np.ascontiguousarray(xs[i]),
            "base": np.ascontiguousarray(bs[i]),
            "W1": W1, "b1": b1, "W2": W2, "b2": b2, "A": A, "Bm": Bm,
        }
        for i in range(N_CORES)
    ]
    res = run_bass_kernel_spmd(nc, in_maps, list(range(N_CORES))).results
    out = np.stack([res[i]["out"] for i in range(N_CORES)], axis=0)
    return out.reshape(B, S, H).astype(np.float32)
